# revision 36
# baseline (speedup 1.0000x reference)
"""Trainium2 Bass kernel for nn_CNNVectorForm (LeNet-style CNN, batch 8192).

Pipeline per core (data-parallel over batch, 1024 images/core):
  conv 5x5 VALID (1->20ch, 28->24)  -> 2x2 maxpool -> fc1(2880->500) + relu
  -> fc2(500->10) + softmax

Device formulation (v2):
  * All activations feature-major [features, batch]; batch rides the free
    dim (512 per tile).
  * Conv inputs are host-im2col'd into K=128 row-groups: 8 input rows x
    16 cols per gather (bf16), each serving 4 output rows x 12 cols.
    One [128, nb] gather feeds 8 matmuls (4 rows x even/odd cols), so
    the input stream is ~2x the raw image (vs 4.9x for the old K=80
    layout) and bf16 halves it again: 3.1 MB/core vs 15.7.
  * Conv weights are a bf16 Toeplitz matrix T8 [128, 8*120]; each matmul
    produces [20ch x 6cols, batch] for one (row-offset, col-parity).
  * 2x2 maxpool: scalar engine copies two PSUM quarters to SBUF, DVE does
    the two leaf maxes (SBUF vs PSUM, bf16 out) and the final max in the
    DVE 16-bit fast mode.
  * fc1 in bf16 (weights host-permuted to pooled-feature order), 24
    accumulating K=120 matmuls per 125-neuron M-tile, skewed behind conv
    by 2 gather-pairs so conv and fc1 matmuls interleave gap-free on PE.
  * 32 dummy matmuls at kernel start (no DMA deps) warm the PE clock
    gate (HAM) to 8/8 before the first real conv matmul; 3 filler
    matmuls per pool round in the first two (fc1-less) gather-pairs keep
    the busy streak unbroken so HAM never re-throttles during ramp-up.
  * startup DMA shaped for the contended 8-core window: sync ring is
    pure gathers, t8 split in halves on the scalar ring, and the 2.9MB
    fc1 weight burst is gated behind t8's arrival on the gpsimd ring.
  * conv bias folded into fc1 bias on the host.
  * fc2 (fp32r) outputs [10, batch]; device applies exp only.  The fc2
    bias and the softmax normalization are folded to the host:
    softmax(z + b2) = (e^z * e^b2) / sum(...)  -- so no PE transposes and
    no DVE reciprocal work on device.
"""

import numpy as np
import ml_dtypes

N, H, W = 8192, 28, 28
COUT, KS = 20, 5
NCORES = 8
NPC = N // NCORES  # images per core
FC1_IN, FC1_OUT, FC2_OUT = 2880, 500, 10
MT, MTS = 4, 125   # fc1 M tiles
KB, KBS = 24, 120  # a1 feature blocks (one per (pooled row, column half))
NG = 6             # conv row groups (4 output rows each)

_cache = {}


def _build(npc, nb):
    from contextlib import ExitStack

    import concourse.tile as tile
    from concourse import bacc, mybir

    f32 = mybir.dt.float32
    f32r = mybir.dt.float32r
    bf16 = mybir.dt.bfloat16
    nbt = npc // nb

    nc = bacc.Bacc(
        "TRN2",
        target_bir_lowering=False,
        debug=False,
        enable_asserts=False,
        num_devices=NCORES,
    )

    # host-im2col'd input: xg[g, jb, p, b] = x[(4g + p//16)*28 + 12*jb + p%16, b]
    xg_d = nc.dram_tensor(
        "xg", [NG, 2, 128, npc], bf16, kind="ExternalInput"
    ).ap()
    t_d = nc.dram_tensor("tmat", [128, 8 * KBS], bf16, kind="ExternalInput").ap()
    w1_d = nc.dram_tensor(
        "w1", [KB // 4, KBS, 4 * FC1_OUT], bf16, kind="ExternalInput"
    ).ap()
    b1_d = nc.dram_tensor("b1", [MTS, MT], f32, kind="ExternalInput").ap()
    w2_d = nc.dram_tensor("w2", [MTS, MT * FC2_OUT], f32r, kind="ExternalInput").ap()
    o_d = nc.dram_tensor("out", [nbt, FC2_OUT, nb], f32, kind="ExternalOutput").ap()

    with tile.TileContext(nc) as tc, ExitStack() as ctx:
        const = ctx.enter_context(tc.tile_pool(name="const", bufs=1))
        w1pool = ctx.enter_context(tc.tile_pool(name="w1", bufs=6))
        gpool = ctx.enter_context(tc.tile_pool(name="gather", bufs=8))
        a1pool = ctx.enter_context(tc.tile_pool(name="a1", bufs=8))
        tmppool = ctx.enter_context(tc.tile_pool(name="ptmp", bufs=8))
        a2pool = ctx.enter_context(tc.tile_pool(name="a2", bufs=2 * MT))
        smpool = ctx.enter_context(tc.tile_pool(name="softmax", bufs=4))
        cpsum = ctx.enter_context(tc.tile_pool(name="cpsum", bufs=4, space="PSUM"))
        fpsum = ctx.enter_context(tc.tile_pool(name="fpsum", bufs=4, space="PSUM"))

        # HAM warm-up: ~36 dummy matmuls with no DMA dependency keep the PE
        # continuously busy from the end of the framework preamble, so the
        # clock gate reaches 8/8 (2.4 GHz) before the first real conv
        # matmul instead of ~15us into the run.  Output is never read.
        warm = const.tile([128, 128], bf16)
        nc.gpsimd.memset(warm[:], 0.0)
        zt = const.tile([MTS, 512], f32)
        nc.gpsimd.memset(zt[:], 0.0)
        wfil = fpsum.tile([128, nb], f32, tag="fps", name="warm")
        wn = min(128, nb)
        for _ in range(32):
            nc.tensor.matmul(wfil[:, :wn], warm[:], warm[:, :wn], start=True,
                             stop=True)

        # t8 rides the scalar ring (split so the first LDWEIGHTS only waits
        # on the first half); the sync ring is pure gathers.  All bulk
        # weight traffic is queued BEHIND t8 on the same ring (ring FIFO),
        # so during the contended 8-core startup window only t8+gather0
        # compete for HBM.
        t8 = const.tile([128, 8 * KBS], bf16)
        nc.scalar.dma_start(t8[:, : 4 * KBS], t_d[:, : 4 * KBS])
        nc.scalar.dma_start(t8[:, 4 * KBS :], t_d[:, 4 * KBS :])
        b1t = const.tile([MTS, MT], f32)
        nc.scalar.dma_start(b1t[:], b1_d[:])
        w2t = const.tile([MTS, MT * FC2_OUT], f32r)
        nc.scalar.dma_start(w2t[:], w2_d[:])
        # fc1 weights: 6 grouped DMAs, host-packed so every group is one
        # fully-contiguous [120, 2000] transfer.  Issued from the idle
        # gpsimd queue, but gated behind t8's arrival via a 1-column copy
        # that reads t8: during the contended 8-core startup window only
        # t8+gather0 compete for HBM, so the first real matmul is never
        # starved (a late start resets the HAM warm-up, costing 2-4us).
        tgate = const.tile([128, 1], bf16)
        nc.gpsimd.tensor_copy(tgate[:], t8[:, 8 * KBS - 1 : 8 * KBS])
        w1g = []
        for gidx in range(KB // 4):
            wt = w1pool.tile([KBS, 4 * FC1_OUT], bf16, tag="w1",
                             name=f"w1g{gidx}")
            nc.gpsimd.dma_start(wt[:], w1_d[gidx])
            w1g.append(wt)

        def w1_slice(j, mt):
            return w1g[j // 4][
                :, (j % 4) * FC1_OUT + mt * MTS : (j % 4) * FC1_OUT + (mt + 1) * MTS
            ]

        SKEW = 2  # pairs (of 2 a1 blocks each) the fc1 stream trails conv by

        for bt in range(nbt):
            b0 = bt * nb
            # fc1 accumulators for all 4 M-tiles ride along with the conv
            # loop: 4 dependency-free fc1 matmuls interleave after each
            # conv quad to keep the PE gap-free.
            fp = [
                fpsum.tile([MTS, nb], f32, tag="fps", name=f"fp{bt}_{mt}")
                for mt in range(MT)
            ]
            a1blk = [None] * KB
            nfc = 0  # fc1 blocks consumed

            def fc1_block(j):
                nonlocal nfc
                for mt in range(MT):
                    nc.tensor.matmul(
                        fp[mt][:],
                        w1_slice(j, mt),
                        a1blk[j][:],
                        start=(nfc == 0),
                        stop=(nfc == KB - 1),
                    )
                nfc += 1

            for u in range(2 * NG + SKEW):
                if u < 2 * NG:
                    g, jb = u >> 1, u & 1
                    gt = gpool.tile([128, nb], bf16, tag="g")
                    nc.sync.dma_start(gt[:], xg_d[g, jb, :, b0 : b0 + nb])
                    for il in range(2):
                        ps = [
                            cpsum.tile([KBS, nb], f32, tag="cps", name=f"cps{i}")
                            for i in range(4)
                        ]
                        for dr in range(2):
                            for eo in range(2):
                                s = (2 * il + dr) * 2 + eo
                                nc.tensor.matmul(
                                    ps[2 * dr + eo][:],
                                    t8[:, s * KBS : (s + 1) * KBS],
                                    gt[:],
                                    start=True,
                                    stop=True,
                                )
                        # leaf maxes read fp32 PSUM but emit bf16, so the
                        # final max runs in the DVE 16-bit fast mode and
                        # fc1 streams bf16 (halving w1 DMA too).
                        s0 = tmppool.tile([KBS, nb], f32, tag="s")
                        nc.scalar.copy(s0[:], ps[0][:])
                        m0 = tmppool.tile([KBS, nb], bf16, tag="m")
                        nc.vector.tensor_max(m0[:], s0[:], ps[1][:])
                        s1 = tmppool.tile([KBS, nb], f32, tag="s")
                        nc.scalar.copy(s1[:], ps[2][:])
                        m1 = tmppool.tile([KBS, nb], bf16, tag="m")
                        nc.vector.tensor_max(m1[:], s1[:], ps[3][:])
                        ab = a1pool.tile([KBS, nb], bf16, tag="a1")
                        nc.vector.tensor_max(ab[:], m0[:], m1[:])
                        a1blk[(2 * g + il) * 2 + jb] = ab
                        # fc1 for one block of the pair SKEW behind us
                        if u >= SKEW:
                            pg, pjb = (u - SKEW) >> 1, (u - SKEW) & 1
                            fc1_block((2 * pg + il) * 2 + pjb)
                        elif bt == 0:
                            # no fc1 yet in the first SKEW pairs: filler
                            # matmuls keep the PE continuously busy through
                            # the pool-chain bubbles so the HAM clock gate
                            # un-throttles ~8us earlier.
                            for _ in range(3):
                                nc.tensor.matmul(
                                    wfil[:], warm[:], gt[:],
                                    start=True, stop=True,
                                )
                else:
                    pg, pjb = (u - SKEW) >> 1, (u - SKEW) & 1
                    for il in range(2):
                        fc1_block((2 * pg + il) * 2 + pjb)

            # a2 eviction split across scalar and vector so the relu drain
            # of the 4 accumulators runs in parallel at the tile tail.
            a2t = [None] * MT
            for mt in range(MT):
                a2 = a2pool.tile([MTS, nb], f32r, tag="a2")
                if mt % 2 == 0:
                    nc.scalar.activation(
                        a2[:],
                        fp[mt][:],
                        mybir.ActivationFunctionType.Relu,
                        bias=b1t[:, mt : mt + 1],
                    )
                else:
                    nc.vector.scalar_tensor_tensor(
                        a2[:],
                        fp[mt][:],
                        b1t[:, mt : mt + 1],
                        zt[:, :nb],
                        mybir.AluOpType.add,
                        mybir.AluOpType.max,
                    )
                a2t[mt] = a2

            # fc2 feature-major: [10, nb] logits; only exp on device --
            # fc2 bias and softmax normalization are folded to the host.
            p2f = fpsum.tile([FC2_OUT, nb], f32, tag="fps", name=f"p2f_{bt}")
            for mt in range(MT):
                nc.tensor.matmul(
                    p2f[:],
                    w2t[:, mt * FC2_OUT : (mt + 1) * FC2_OUT],
                    a2t[mt][:],
                    start=(mt == 0),
                    stop=(mt == MT - 1),
                )
            et = smpool.tile([FC2_OUT, nb], f32, tag="e")
            nc.scalar.activation(
                et[:], p2f[:], mybir.ActivationFunctionType.Exp
            )
            nc.sync.dma_start(o_d[bt], et[:])

    nc.compile()
    return nc


def _prep_weights(conv_w, conv_b, fc1_w, fc1_b, fc2_w, fc2_b):
    conv_w = np.asarray(conv_w, np.float32).reshape(COUT, KS, KS)
    conv_b = np.asarray(conv_b, np.float32)
    fc1_w = np.asarray(fc1_w, np.float32)
    fc1_b = np.asarray(fc1_b, np.float32)
    fc2_w = np.asarray(fc2_w, np.float32)

    # Toeplitz conv matrix [128, 8*120]: row p = di8*16 + jjp (input row
    # offset within the 8-row group, input col within the 16-wide block);
    # slice s = ro*2 + eo (output row offset 0..3, col parity); within a
    # slice, col m = ch*6 + q for output col jj = 2q + eo.
    T = np.zeros((128, 8 * KBS), np.float32)
    for s in range(8):
        ro, eo = s >> 1, s & 1
        for m in range(KBS):
            ch, q = m // 6, m % 6
            jj = 2 * q + eo
            for di in range(KS):
                for dj in range(KS):
                    T[(ro + di) * 16 + jj + dj, s * KBS + m] = conv_w[ch, di, dj]

    # fc1 weights permuted to our pooled-feature order:
    # block kb = ip*2 + jb, within-block m = ch*6 + q
    # -> original flat feature ch*144 + ip*12 + jb*6 + q
    kbv = np.arange(KB)
    ipv, jbv = kbv // 2, kbv % 2
    ml = np.arange(KBS)
    cv, qv = ml // 6, ml % 6
    fidx = cv[None, :] * 144 + ipv[:, None] * 12 + jbv[:, None] * 6 + qv[None, :]
    w1 = fc1_w.T[fidx.reshape(-1)].reshape(KB, KBS, FC1_OUT)
    # pack into 6 contiguous groups of 4 blocks: [6, 120, 4*500]
    w1 = np.ascontiguousarray(
        w1.reshape(KB // 4, 4, KBS, FC1_OUT).transpose(0, 2, 1, 3)
    ).reshape(KB // 4, KBS, 4 * FC1_OUT).astype(ml_dtypes.bfloat16)

    # conv bias folded into fc1 bias (pool-max commutes with per-channel const)
    cb_vec = np.repeat(conv_b, 144)
    b1p = fc1_b + fc1_w @ cb_vec
    b1 = np.ascontiguousarray(b1p.reshape(MT, MTS).T)

    w2 = np.ascontiguousarray(
        fc2_w.T.reshape(MT, MTS, FC2_OUT).transpose(1, 0, 2)
    ).reshape(MTS, MT * FC2_OUT)
    return (T.astype(ml_dtypes.bfloat16), w1, b1, w2)


# im2col pixel indices: idx[g, jb, di8*16+jjp] = (4g+di8)*28 + 12*jb + jjp
_IDX = np.zeros((NG, 2, 128), np.int64)
for _g in range(NG):
    for _jb in range(2):
        for _di in range(8):
            for _jjp in range(16):
                _IDX[_g, _jb, _di * 16 + _jjp] = (4 * _g + _di) * W + 12 * _jb + _jjp


def _prep_x(x_core):
    """x_core [784, npc] pixel-major fp32 -> xg [NG, 2, 128, npc] bf16."""
    return np.ascontiguousarray(
        x_core[_IDX.reshape(-1)].reshape(NG, 2, 128, x_core.shape[1])
    ).astype(ml_dtypes.bfloat16)


def _postprocess(e_cores, fc2_b):
    """e_cores: list of [nbt, 10, nb] exp(logit) arrays -> [N, 10] softmax."""
    e = np.concatenate(
        [np.asarray(ec, np.float32).transpose(0, 2, 1).reshape(-1, FC2_OUT)
         for ec in e_cores], axis=0)
    e = e * np.exp(np.asarray(fc2_b, np.float32))[None, :]
    return e / e.sum(axis=1, keepdims=True)


def _run(inputs, npc=NPC, nb=512, trace=False):
    from concourse import bass_utils

    key = (npc, nb)
    if key not in _cache:
        _cache[key] = _build(npc, nb)
    nc = _cache[key]

    T, w1, b1, w2 = _prep_weights(
        inputs["conv_w"], inputs["conv_b"], inputs["fc1_w"],
        inputs["fc1_b"], inputs["fc2_w"], inputs["fc2_b"],
    )
    x = np.asarray(inputs["x"], np.float32).reshape(-1, H * W)
    n_total = x.shape[0]
    assert n_total == NCORES * npc
    xs = x.reshape(NCORES, npc, H * W).transpose(0, 2, 1)

    in_maps = [
        {"xg": _prep_x(xs[i]), "tmat": T, "w1": w1, "b1": b1, "w2": w2}
        for i in range(NCORES)
    ]
    res = bass_utils.run_bass_kernel_spmd(
        nc, in_maps, core_ids=list(range(NCORES)), trace=trace
    )
    out = _postprocess(
        [res.results[i]["out"] for i in range(NCORES)], inputs["fc2_b"]
    )
    return out, res


def kernel(**inputs):
    out, _ = _run(inputs)
    return out
